# revision 12
# baseline (speedup 1.0000x reference)
"""Trainium2 Bass kernel for nn_ConvShare: multi-width causal conv + shared projection.

Reference computation (per batch element b):
    xpad = pad(x[b], L -> L+W-1)                       # [L+11, D]
    taps[k]  = xpad[k:k+L, :] @ conv_w[:, :, k].T      # [L, D], k = 0..W-1
    spans[k] = cumsum_k taps                           # [L, D]
    h[k]     = relu(spans[k])
    out[:, k, :] = h[k] @ proj_w.T + proj_b            # [L, W, D]

Sharding: data-parallel over batch B=8 across the 8 NeuronCores (no
communication; conv_w/proj_w replicated per core).

Everything on-chip is feature-major ([D, L], contraction dim on SBUF
partitions). fp16 matmul inputs (PSUM accumulation fp32; conv cumsum
carried in fp32 SBUF). The PE roofline is 24 unit-matmuls of
512x768x768 = 184.3us; this implementation closes in on it with:
  - PE warm-up: dummy N=32 matmuls with no DMA deps run during the
    startup input DMA so the HAM clock gate (1.2 -> 2.4 GHz) opens
    before the first real matmul.
  - c-outer conv waves + finely split tap-0 DMAs (xT in 4 partition
    slices, tap-0 conv weights in 36 [128,128] pieces) so the first
    conv wave needs only ~330KB of DMA instead of ~2MB.
  - Batched DMA everywhere else: one 9KB-per-partition transfer per
    conv-weight tap, one 6KB-per-partition transfer per output tap
    (128 lines instead of 768). The DMA engines are line-issue-rate
    limited (~0.1-0.2us per line per queue), not bandwidth limited.
  - fp16 output, transposed/upcast to [L, W, D] host-side.
  - The last tap is emitted in two l-halves and its output DMA in two
    o2-halves so the final drain chain is short.
"""

import os
import sys

import numpy as np

if True:  # make concourse importable regardless of harness cwd
    for _p in ("/opt/trn_rl_repo", "/opt/pypackages"):
        if _p not in sys.path and os.path.isdir(_p):
            sys.path.append(_p)

from contextlib import ExitStack  # noqa: E402

import concourse.bacc as bacc  # noqa: E402
import concourse.bass as bass  # noqa: E402
import concourse.mybir as mybir  # noqa: E402
import concourse.tile as tile  # noqa: E402
from concourse import bass_utils  # noqa: E402

B, L, D, W = 8, 512, 768, 12
P = 128          # SBUF partitions
C = D // P       # 6 contraction chunks of 128
LP = L + W - 1   # 523: right-padded sequence length

F32 = mybir.dt.float32
F16 = mybir.dt.float16
RELU = mybir.ActivationFunctionType.Relu

WARMUP = 0       # dummy matmuls to open the HAM clock gate during startup DMA.
#                  Measured harmful: the gpsimd memsets they depend on only
#                  execute ~7us in (engine spin-up), so the dummies delay the
#                  real stream; the cold-start penalty they would save hides
#                  under tap-0's DMA pacing anyway.
SPLIT_LAST = True  # emit tap W-1 in two l-halves for a faster tail drain

# Knobs the test harness may flip before calling kernel():
TRACE = False
LAST_RESULTS = None


def _build_program(warmup: int, split_last: bool) -> bass.Bass:
    mdt = F16

    nc = bacc.Bacc(
        "TRN2",
        target_bir_lowering=False,
        debug=False,
        num_devices=B,
    )

    # DRAM I/O (pre-arranged host-side so every DMA is a clean copy with
    # large contiguous per-partition lines).
    xT = nc.dram_tensor("xT", [C, P, LP], mdt, kind="ExternalInput").ap()
    cw0s = nc.dram_tensor("cw0s", [C, P, D], mdt, kind="ExternalInput").ap()
    cwB = nc.dram_tensor("cwB", [W, P, C * D], mdt, kind="ExternalInput").ap()
    pwB = nc.dram_tensor("pwB", [P, C * D], mdt, kind="ExternalInput").ap()
    pb = nc.dram_tensor("pb", [C, P, 1], F32, kind="ExternalInput").ap()
    out = nc.dram_tensor("out", [W, P, C, L], F16, kind="ExternalOutput").ap()

    with tile.TileContext(nc) as tc, ExitStack() as ctx:
        const_pool = ctx.enter_context(tc.tile_pool(name="const", bufs=1))
        cw_pool = ctx.enter_context(tc.tile_pool(name="cw", bufs=2))
        h_pool = ctx.enter_context(tc.tile_pool(name="h", bufs=2))
        out_pool = ctx.enter_context(tc.tile_pool(name="out", bufs=2))
        psc_pool = ctx.enter_context(tc.tile_pool(name="psc", bufs=1, space="PSUM"))
        psp_pool = ctx.enter_context(tc.tile_pool(name="psp", bufs=2, space="PSUM"))

        # --- PE warm-up: no-data-dependency matmuls that run while the first
        # input DMAs are in flight, so the HAM gate opens (~3.4us of PE busy)
        # before the real stream starts.
        if warmup:
            wa = const_pool.tile([P, P], mdt, name="warm_a")
            wb = const_pool.tile([P, 512], mdt, name="warm_b")
            nc.gpsimd.memset(wa[:], 0.0)
            nc.gpsimd.memset(wb[:], 0.0)
            # short MMs to accumulate ~2.5us of PE-busy quickly, then a few
            # N=512 ones to stay busy until the first real matmul's inputs
            # land (a >3.4us idle gap would re-throttle the HAM gate).
            for wi in range(warmup):
                wp = psp_pool.tile([P, 512], F32, tag="psp", name=f"warm_ps{wi}")
                nn = 32 if wi < warmup - 10 else 512
                nc.tensor.matmul(
                    wp[:, 0:nn], lhsT=wa[:], rhs=wb[:, 0:nn], start=True, stop=True
                )

        # --- running conv cumsum, fp32 (memset overlaps startup DMA)
        spans = const_pool.tile([P, C * L], F32)
        nc.gpsimd.memset(spans[:], 0.0)

        # --- startup DMAs in "wave" order: the tap-0 conv runs c-outer, so
        # wave c needs only xT[c] (4 partition-sliced DMAs) + 6 small weight
        # pieces. pw/pb/cw[1] follow after tap-0's needs.
        xT_t = []
        cw0_t = []  # [c] -> [128, D] tile (tap-0 weights, one DMA per chunk:
        #             keep startup lines at 1-1.5KB — finer splits explode the
        #             DMA line count and crawl for ~50us at ~150ns/line/queue)
        for c in range(C):
            xt = const_pool.tile([P, LP], mdt, tag=f"xt{c}", name=f"xt{c}")
            nc.sync.dma_start(xt[:], xT[c, :, :])
            xT_t.append(xt)
            t = const_pool.tile([P, D], mdt, tag=f"cw0_{c}", name=f"cw0_{c}")
            nc.sync.dma_start(t[:], cw0s[c, :, :])
            cw0_t.append(t)

        def load_cw(k):
            # one DMA per tap: 128 lines of 9KB
            t = cw_pool.tile([P, C * D], mdt, tag="cw", name=f"cw_{k}")
            nc.sync.dma_start(t[:], cwB[k, :, :])
            return t

        # conv psum banks: one persistent tag per output block (6 banks; the
        # remaining 2 banks cycle for the proj).
        ps = [
            psc_pool.tile([P, L], F32, tag=f"sp{ob}", name=f"sp{ob}")
            for ob in range(C)
        ]

        def conv_tap(k, cw_k, l0, ln):
            # wave order: all 6 output blocks for contraction chunk c, then c+1.
            for c in range(C):
                for ob in range(C):
                    lhsT = (
                        cw0_t[c][:, ob * P : (ob + 1) * P]
                        if k == 0
                        else cw_k[:, c * D + ob * P : c * D + (ob + 1) * P]
                    )
                    nc.tensor.matmul(
                        ps[ob][:, l0 : l0 + ln],
                        lhsT=lhsT,
                        rhs=xT_t[c][:, k + l0 : k + l0 + ln],
                        start=(c == 0),
                        stop=(c == C - 1),
                        skip_group_check=True,
                    )

        def relu_tap(h_t, l0, ln):
            for ob in range(C):
                sp = spans[:, ob * L + l0 : ob * L + l0 + ln]
                nc.vector.tensor_add(sp, sp, ps[ob][:, l0 : l0 + ln])  # cumsum
                nc.scalar.activation(h_t[ob][:, l0 : l0 + ln], sp, RELU)

        def proj_tap(k, h_t, pw_t, pb_t, o_tap, l0, ln):
            for o2b in range(C):
                pp = psp_pool.tile([P, 512], F32, tag="psp", name=f"pp_{k}_{o2b}_{l0}")
                for c in range(C):
                    nc.tensor.matmul(
                        pp[:, 0:ln],
                        lhsT=pw_t[:, c * D + o2b * P : c * D + (o2b + 1) * P],
                        rhs=h_t[c][:, l0 : l0 + ln],
                        start=(c == 0),
                        stop=(c == C - 1),
                    )
                nc.vector.tensor_scalar_add(
                    o_tap[:, o2b * L + l0 : o2b * L + l0 + ln], pp[:, 0:ln], pb_t[o2b][:]
                )

        # --- tap 0 conv first (its matmuls only need the wave DMAs above) ---
        conv_tap(0, None, 0, L)

        # remaining startup loads, ordered by first use
        cw_cur = load_cw(1)
        pw_t = const_pool.tile([P, C * D], mdt, name="pw")
        nc.sync.dma_start(pw_t[:], pwB[:, :])
        pb_t = []
        for c in range(C):
            t = const_pool.tile([P, 1], F32, tag=f"pb{c}", name=f"pb{c}")
            nc.sync.dma_start(t[:], pb[c, :, :])
            pb_t.append(t)

        h_t = [h_pool.tile([P, L], mdt, tag=f"h{c}", name=f"h{c}_0") for c in range(C)]
        o_tap = out_pool.tile([P, C * L], F16, tag="out", name="o_0")
        relu_tap(h_t, 0, L)
        proj_tap(0, h_t, pw_t, pb_t, o_tap, 0, L)
        nc.sync.dma_start(out[0, :, :, :], o_tap[:])

        pend = None  # (h_t, o_tap) of tap W-2, whose proj is deferred so the
        #              last tap's conv+relu can hide under it on the PE
        for k in range(1, W):
            cw_k = cw_cur
            cw_cur = load_cw(k + 1) if k + 1 < W else None
            h_t = [
                h_pool.tile([P, L], mdt, tag=f"h{c}", name=f"h{c}_{k}")
                for c in range(C)
            ]
            o_tap = out_pool.tile([P, C * L], F16, tag="out", name=f"o_{k}")
            conv_tap(k, cw_k, 0, L)
            relu_tap(h_t, 0, L)
            if split_last and k == W - 2:
                pend = (h_t, o_tap)
                continue
            if pend is not None:
                ph, po = pend
                pend = None
                proj_tap(k - 1, ph, pw_t, pb_t, po, 0, L)
                nc.sync.dma_start(out[k - 1, :, :, :], po[:])
            if split_last and k == W - 1:
                # output in two o2-half DMAs (3KB lines): the first drains
                # while proj groups 3-5 still compute.
                for o2b in range(C):
                    pp = psp_pool.tile([P, 512], F32, tag="psp", name=f"pp_{k}_{o2b}")
                    for c in range(C):
                        nc.tensor.matmul(
                            pp[:],
                            lhsT=pw_t[:, c * D + o2b * P : c * D + (o2b + 1) * P],
                            rhs=h_t[c][:],
                            start=(c == 0),
                            stop=(c == C - 1),
                        )
                    nc.vector.tensor_scalar_add(
                        o_tap[:, o2b * L : (o2b + 1) * L], pp[:], pb_t[o2b][:]
                    )
                    if o2b == 2:
                        nc.sync.dma_start(out[k, :, 0:3, :], o_tap[:, 0 : 3 * L])
                    elif o2b == 5:
                        nc.sync.dma_start(out[k, :, 3:6, :], o_tap[:, 3 * L : 6 * L])
            else:
                proj_tap(k, h_t, pw_t, pb_t, o_tap, 0, L)
                nc.sync.dma_start(out[k, :, :, :], o_tap[:])

    nc.compile()
    return nc


_program_cache: dict = {}


def _get_program() -> bass.Bass:
    key = (WARMUP, SPLIT_LAST)
    if key not in _program_cache:
        _program_cache[key] = _build_program(WARMUP, SPLIT_LAST)
    return _program_cache[key]


def _prep_inputs(x, conv_w, proj_w, proj_b):
    x = np.asarray(x, dtype=np.float32)
    conv_w = np.asarray(conv_w, dtype=np.float32)
    proj_w = np.asarray(proj_w, dtype=np.float32)
    proj_b = np.asarray(proj_b, dtype=np.float32)

    xT_all = np.zeros((B, D, LP), dtype=np.float32)              # [B, D, L+W-1]
    xT_all[:, :, :L] = x.transpose(0, 2, 1)
    xT_all = np.ascontiguousarray(xT_all.reshape(B, C, P, LP).astype(np.float16))
    cwT = conv_w.transpose(2, 1, 0).reshape(W, C, P, D).astype(np.float16)
    # tap-0 weights separately in per-chunk layout (startup waves)
    cw0s = np.ascontiguousarray(cwT[0])                          # [C, P, D]
    # batched per-tap layout: cwB[k, p, c*D + o] = conv_w[o, c*128+p, k]
    cwB = np.ascontiguousarray(cwT.transpose(0, 2, 1, 3).reshape(W, P, C * D))
    pwT = proj_w.T.reshape(C, P, D).astype(np.float16)
    pwB = np.ascontiguousarray(pwT.transpose(1, 0, 2).reshape(P, C * D))
    pbb = np.ascontiguousarray(proj_b.reshape(C, P, 1))
    return xT_all, cw0s, cwB, pwB, pbb


def kernel(x, conv_w, proj_w, proj_b):
    global LAST_RESULTS
    nc = _get_program()
    xT_all, cw0s, cwB, pwB, pbb = _prep_inputs(x, conv_w, proj_w, proj_b)
    in_maps = [
        {"xT": xT_all[b], "cw0s": cw0s, "cwB": cwB, "pwB": pwB, "pb": pbb}
        for b in range(B)
    ]
    res = bass_utils.run_bass_kernel_spmd(
        nc, in_maps, core_ids=list(range(B)), trace=TRACE
    )
    LAST_RESULTS = res
    # per-core out is [W, P, C, L] f16; final layout is [L, W, D] with
    # D = c*128 + p
    return np.stack(
        [
            np.ascontiguousarray(
                r["out"].astype(np.float32).transpose(3, 0, 2, 1).reshape(L, W, D)
            )
            for r in res.results
        ],
        axis=0,
    )


# revision 15
# speedup vs baseline: 1.1914x; 1.1914x over previous
"""Trainium2 Bass kernel for nn_ConvShare: multi-width causal conv + shared projection.

Reference computation (per batch element b):
    xpad = pad(x[b], L -> L+W-1)                       # [L+11, D]
    taps[k]  = xpad[k:k+L, :] @ conv_w[:, :, k].T      # [L, D], k = 0..W-1
    spans[k] = cumsum_k taps                           # [L, D]
    h[k]     = relu(spans[k])
    out[:, k, :] = h[k] @ proj_w.T + proj_b            # [L, W, D]

Sharding: data-parallel over batch B=8 across the 8 NeuronCores (no
communication; conv_w/proj_w replicated per core).

Everything on-chip is feature-major ([D, L], contraction dim on SBUF
partitions). fp16 matmul inputs (PSUM accumulation fp32; conv cumsum
carried in fp32 SBUF). The PE roofline is 24 unit-matmuls of
512x768x768 = 184.3us; this implementation closes in on it with:
  - PE warm-up: dummy N=32 matmuls with no DMA deps run during the
    startup input DMA so the HAM clock gate (1.2 -> 2.4 GHz) opens
    before the first real matmul.
  - c-outer conv waves + finely split tap-0 DMAs (xT in 4 partition
    slices, tap-0 conv weights in 36 [128,128] pieces) so the first
    conv wave needs only ~330KB of DMA instead of ~2MB.
  - Batched DMA everywhere else: one 9KB-per-partition transfer per
    conv-weight tap, one 6KB-per-partition transfer per output tap
    (128 lines instead of 768). The DMA engines are line-issue-rate
    limited (~0.1-0.2us per line per queue), not bandwidth limited.
  - fp16 output, transposed/upcast to [L, W, D] host-side.
  - The last tap is emitted in two l-halves and its output DMA in two
    o2-halves so the final drain chain is short.
"""

import os
import sys

import numpy as np

if True:  # make concourse importable regardless of harness cwd
    for _p in ("/opt/trn_rl_repo", "/opt/pypackages"):
        if _p not in sys.path and os.path.isdir(_p):
            sys.path.append(_p)

from contextlib import ExitStack  # noqa: E402

import concourse.bacc as bacc  # noqa: E402
import concourse.bass as bass  # noqa: E402
import concourse.mybir as mybir  # noqa: E402
import concourse.tile as tile  # noqa: E402
from concourse import bass_utils  # noqa: E402

B, L, D, W = 8, 512, 768, 12
P = 128          # SBUF partitions
C = D // P       # 6 contraction chunks of 128
LP = L + W - 1   # 523: right-padded sequence length

F32 = mybir.dt.float32
F16 = mybir.dt.float16
RELU = mybir.ActivationFunctionType.Relu

WARMUP = 0       # dummy LDWEIGHTS to open the HAM clock gate during startup
#                  DMA. Left off: dummy matmuls measured harmful (the memsets
#                  they need only execute ~7us in, delaying the real stream);
#                  bare LDWEIGHTS of a never-written tile is rejected by the
#                  Tile allocator, and any engine that could write one only
#                  spins up at ~5-8us — when the real stream starts anyway.
#                  The ~8 cold matmuls at stream start hide under tap-0's
#                  DMA pacing.
SPLIT_LAST = True  # emit tap W-1 in two l-halves for a faster tail drain

# Knobs the test harness may flip before calling kernel():
TRACE = False
LAST_RESULTS = None


def _build_program(warmup: int, split_last: bool) -> bass.Bass:
    mdt = F16

    nc = bacc.Bacc(
        "TRN2",
        target_bir_lowering=False,
        debug=False,
        num_devices=B,
    )

    # DRAM I/O (pre-arranged host-side so every DMA is a clean copy with
    # large contiguous per-partition lines).
    xT = nc.dram_tensor("xT", [C, P, LP], mdt, kind="ExternalInput").ap()
    cw0s = nc.dram_tensor("cw0s", [C, P, D], mdt, kind="ExternalInput").ap()
    cwB = nc.dram_tensor("cwB", [W, P, C * D], mdt, kind="ExternalInput").ap()
    pwB = nc.dram_tensor("pwB", [P, C * D], mdt, kind="ExternalInput").ap()
    pb = nc.dram_tensor("pb", [C, P, 1], F32, kind="ExternalInput").ap()
    out = nc.dram_tensor("out", [W, P, C, L], F16, kind="ExternalOutput").ap()

    with tile.TileContext(nc) as tc, ExitStack() as ctx:
        const_pool = ctx.enter_context(tc.tile_pool(name="const", bufs=1))
        cw_pool = ctx.enter_context(tc.tile_pool(name="cw", bufs=2))
        h_pool = ctx.enter_context(tc.tile_pool(name="h", bufs=2))
        out_pool = ctx.enter_context(tc.tile_pool(name="out", bufs=2))
        psc_pool = ctx.enter_context(tc.tile_pool(name="psc", bufs=1, space="PSUM"))
        psp_pool = ctx.enter_context(tc.tile_pool(name="psp", bufs=2, space="PSUM"))

        # --- PE warm-up: no-data-dependency matmuls that run while the first
        # input DMAs are in flight, so the HAM gate opens (~3.4us of PE busy)
        # before the real stream starts.
        if warmup:
            # Bare LDWEIGHTS as PE warm-up: no memset or PSUM dependency, so
            # they issue at ~0.3us (engine spin-up) and accumulate PE-busy
            # (~107ns each) to open the HAM clock gate before the first real
            # matmul's inputs land. Loaded garbage is overwritten by the
            # first real matmul's own weight load.
            wa = const_pool.tile([P, P], mdt, name="warm_a")
            for wi in range(warmup):
                nc.tensor.ldweights(weights=wa[:])

        # --- running conv cumsum, fp32 (memset overlaps startup DMA)
        spans = const_pool.tile([P, C * L], F32)
        nc.gpsimd.memset(spans[:], 0.0)

        # --- startup DMAs in "wave" order: the tap-0 conv runs c-outer, so
        # wave c needs only xT[c] (4 partition-sliced DMAs) + 6 small weight
        # pieces. pw/pb/cw[1] follow after tap-0's needs.
        xT_t = []
        cw0_t = []  # [c] -> [128, D] tile (tap-0 weights, one DMA per chunk:
        #             keep startup lines at 1-1.5KB — finer splits explode the
        #             DMA line count and crawl for ~50us at ~150ns/line/queue)
        for c in range(C):
            xt = const_pool.tile([P, LP], mdt, tag=f"xt{c}", name=f"xt{c}")
            nc.sync.dma_start(xt[:], xT[c, :, :])
            xT_t.append(xt)
            t = const_pool.tile([P, D], mdt, tag=f"cw0_{c}", name=f"cw0_{c}")
            nc.sync.dma_start(t[:], cw0s[c, :, :])
            cw0_t.append(t)

        def load_cw(k):
            # one DMA per tap: 128 lines of 9KB
            t = cw_pool.tile([P, C * D], mdt, tag="cw", name=f"cw_{k}")
            nc.sync.dma_start(t[:], cwB[k, :, :])
            return t

        # conv psum banks: one persistent tag per output block (6 banks; the
        # remaining 2 banks cycle for the proj).
        ps = [
            psc_pool.tile([P, L], F32, tag=f"sp{ob}", name=f"sp{ob}")
            for ob in range(C)
        ]

        def conv_tap(k, cw_k, l0, ln):
            # wave order: all 6 output blocks for contraction chunk c, then c+1.
            for c in range(C):
                for ob in range(C):
                    lhsT = (
                        cw0_t[c][:, ob * P : (ob + 1) * P]
                        if k == 0
                        else cw_k[:, c * D + ob * P : c * D + (ob + 1) * P]
                    )
                    nc.tensor.matmul(
                        ps[ob][:, l0 : l0 + ln],
                        lhsT=lhsT,
                        rhs=xT_t[c][:, k + l0 : k + l0 + ln],
                        start=(c == 0),
                        stop=(c == C - 1),
                        skip_group_check=True,
                    )

        def relu_tap(h_t, l0, ln):
            for ob in range(C):
                sp = spans[:, ob * L + l0 : ob * L + l0 + ln]
                nc.vector.tensor_add(sp, sp, ps[ob][:, l0 : l0 + ln])  # cumsum
                nc.scalar.activation(h_t[ob][:, l0 : l0 + ln], sp, RELU)

        def proj_tap(k, h_t, pw_t, pb_t, o_tap, l0, ln):
            for o2b in range(C):
                pp = psp_pool.tile([P, 512], F32, tag="psp", name=f"pp_{k}_{o2b}_{l0}")
                for c in range(C):
                    nc.tensor.matmul(
                        pp[:, 0:ln],
                        lhsT=pw_t[:, c * D + o2b * P : c * D + (o2b + 1) * P],
                        rhs=h_t[c][:, l0 : l0 + ln],
                        start=(c == 0),
                        stop=(c == C - 1),
                    )
                nc.vector.tensor_scalar_add(
                    o_tap[:, o2b * L + l0 : o2b * L + l0 + ln], pp[:, 0:ln], pb_t[o2b][:]
                )

        # --- tap 0 conv first (its matmuls only need the wave DMAs above) ---
        conv_tap(0, None, 0, L)

        # remaining startup loads, ordered by first use
        cw_cur = load_cw(1)
        pw_t = const_pool.tile([P, C * D], mdt, name="pw")
        nc.sync.dma_start(pw_t[:], pwB[:, :])
        pb_t = []
        for c in range(C):
            t = const_pool.tile([P, 1], F32, tag=f"pb{c}", name=f"pb{c}")
            nc.sync.dma_start(t[:], pb[c, :, :])
            pb_t.append(t)

        h_t = [h_pool.tile([P, L], mdt, tag=f"h{c}", name=f"h{c}_0") for c in range(C)]
        o_tap = out_pool.tile([P, C * L], F16, tag="out", name="o_0")
        relu_tap(h_t, 0, L)
        proj_tap(0, h_t, pw_t, pb_t, o_tap, 0, L)
        nc.sync.dma_start(out[0, :, :, :], o_tap[:])

        pend = None  # (h_t, o_tap) of tap W-2, whose proj is deferred so the
        #              last tap's conv+relu can hide under it on the PE
        for k in range(1, W):
            cw_k = cw_cur
            cw_cur = load_cw(k + 1) if k + 1 < W else None
            h_t = [
                h_pool.tile([P, L], mdt, tag=f"h{c}", name=f"h{c}_{k}")
                for c in range(C)
            ]
            o_tap = out_pool.tile([P, C * L], F16, tag="out", name=f"o_{k}")
            conv_tap(k, cw_k, 0, L)
            relu_tap(h_t, 0, L)
            if split_last and k == W - 2:
                pend = (h_t, o_tap)
                continue
            if pend is not None:
                ph, po = pend
                pend = None
                proj_tap(k - 1, ph, pw_t, pb_t, po, 0, L)
                nc.sync.dma_start(out[k - 1, :, :, :], po[:])
            if split_last and k == W - 1:
                # output in two o2-half DMAs (3KB lines): the first drains
                # while proj groups 3-5 still compute.
                for o2b in range(C):
                    pp = psp_pool.tile([P, 512], F32, tag="psp", name=f"pp_{k}_{o2b}")
                    for c in range(C):
                        nc.tensor.matmul(
                            pp[:],
                            lhsT=pw_t[:, c * D + o2b * P : c * D + (o2b + 1) * P],
                            rhs=h_t[c][:],
                            start=(c == 0),
                            stop=(c == C - 1),
                        )
                    nc.vector.tensor_scalar_add(
                        o_tap[:, o2b * L : (o2b + 1) * L], pp[:], pb_t[o2b][:]
                    )
                    if o2b == 2:
                        nc.sync.dma_start(out[k, :, 0:3, :], o_tap[:, 0 : 3 * L])
                    elif o2b == 5:
                        nc.sync.dma_start(out[k, :, 3:6, :], o_tap[:, 3 * L : 6 * L])
            else:
                proj_tap(k, h_t, pw_t, pb_t, o_tap, 0, L)
                nc.sync.dma_start(out[k, :, :, :], o_tap[:])

    nc.compile()
    return nc


_program_cache: dict = {}


def _get_program() -> bass.Bass:
    key = (WARMUP, SPLIT_LAST)
    if key not in _program_cache:
        _program_cache[key] = _build_program(WARMUP, SPLIT_LAST)
    return _program_cache[key]


def _prep_inputs(x, conv_w, proj_w, proj_b):
    x = np.asarray(x, dtype=np.float32)
    conv_w = np.asarray(conv_w, dtype=np.float32)
    proj_w = np.asarray(proj_w, dtype=np.float32)
    proj_b = np.asarray(proj_b, dtype=np.float32)

    xT_all = np.zeros((B, D, LP), dtype=np.float32)              # [B, D, L+W-1]
    xT_all[:, :, :L] = x.transpose(0, 2, 1)
    xT_all = np.ascontiguousarray(xT_all.reshape(B, C, P, LP).astype(np.float16))
    cwT = conv_w.transpose(2, 1, 0).reshape(W, C, P, D).astype(np.float16)
    # tap-0 weights separately in per-chunk layout (startup waves)
    cw0s = np.ascontiguousarray(cwT[0])                          # [C, P, D]
    # batched per-tap layout: cwB[k, p, c*D + o] = conv_w[o, c*128+p, k]
    cwB = np.ascontiguousarray(cwT.transpose(0, 2, 1, 3).reshape(W, P, C * D))
    pwT = proj_w.T.reshape(C, P, D).astype(np.float16)
    pwB = np.ascontiguousarray(pwT.transpose(1, 0, 2).reshape(P, C * D))
    pbb = np.ascontiguousarray(proj_b.reshape(C, P, 1))
    return xT_all, cw0s, cwB, pwB, pbb


def kernel(x, conv_w, proj_w, proj_b):
    global LAST_RESULTS
    nc = _get_program()
    xT_all, cw0s, cwB, pwB, pbb = _prep_inputs(x, conv_w, proj_w, proj_b)
    in_maps = [
        {"xT": xT_all[b], "cw0s": cw0s, "cwB": cwB, "pwB": pwB, "pb": pbb}
        for b in range(B)
    ]
    res = bass_utils.run_bass_kernel_spmd(
        nc, in_maps, core_ids=list(range(B)), trace=TRACE
    )
    LAST_RESULTS = res
    # per-core out is [W, P, C, L] f16; final layout is [L, W, D] with
    # D = c*128 + p
    return np.stack(
        [
            np.ascontiguousarray(
                r["out"].astype(np.float32).transpose(3, 0, 2, 1).reshape(L, W, D)
            )
            for r in res.results
        ],
        axis=0,
    )


# revision 17
# speedup vs baseline: 1.2033x; 1.0100x over previous
"""Trainium2 Bass kernel for nn_ConvShare: multi-width causal conv + shared projection.

Reference computation (per batch element b):
    xpad = pad(x[b], L -> L+W-1)                       # [L+11, D]
    taps[k]  = xpad[k:k+L, :] @ conv_w[:, :, k].T      # [L, D], k = 0..W-1
    spans[k] = cumsum_k taps                           # [L, D]
    h[k]     = relu(spans[k])
    out[:, k, :] = h[k] @ proj_w.T + proj_b            # [L, W, D]

Sharding: data-parallel over batch B=8 across the 8 NeuronCores (no
communication; conv_w/proj_w replicated per core).

On-chip layout is feature-major ([D, L], contraction dim on SBUF
partitions) for the conv stage; the proj stage uses h as the stationary
matmul operand so its output lands row-major [L, D] and DMAs straight
into the final [L, W, D] layout with 3KB contiguous bursts.

MODE selects the matmul input dtype (PSUM accumulation is fp32 in all
modes; the conv cumsum is carried in fp32):
  - "f16" (default): fp16 inputs. Full PE rate (1 cycle/row) with fast
    weight load; ~209us/core, rel err ~4e-4. Value ranges here (|x|<~6,
    |w|<0.04, |h|<~8) are far inside fp16 range.
  - "f32r": full fp32 data in the fast fp32 PE mode. Most accurate
    (~2e-4) but each matmul pays a ~227ns 4-byte LDWEIGHTS -> ~282us.
  - "bf16": same speed as f16 but ~8x worse rounding (~3.5e-3).
"""

import os
import sys

import numpy as np

if True:  # make concourse importable regardless of harness cwd
    for _p in ("/opt/trn_rl_repo", "/opt/pypackages"):
        if _p not in sys.path and os.path.isdir(_p):
            sys.path.append(_p)

from contextlib import ExitStack  # noqa: E402

import ml_dtypes  # noqa: E402

import concourse.bacc as bacc  # noqa: E402
import concourse.bass as bass  # noqa: E402
import concourse.mybir as mybir  # noqa: E402
import concourse.tile as tile  # noqa: E402
from concourse import bass_utils  # noqa: E402

B, L, D, W = 8, 512, 768, 12
P = 128          # SBUF partitions
C = D // P       # 6 contraction chunks of 128
LP = L + W - 1   # 523: right-padded sequence length
NB = L // P      # 4 output row blocks for proj

F32 = mybir.dt.float32
RELU = mybir.ActivationFunctionType.Relu

MODE = "f16"     # "f32r" | "bf16" | "f16"
CUMSUM = "sbuf"  # "sbuf" | "psum"
STRUCT = "lmajor"  # proj output layout: "lmajor" ([l,o2], direct DMA) | "fmajor" ([o2,l], host transpose)
WARMUP = 0       # fp16 HAM warm-up matmuls; measured no gain (ramp hides under startup DMA), keep off

# Knobs the test harness may flip before calling kernel():
TRACE = False
LAST_RESULTS = None


def _build_program(mode: str, cumsum: str = "sbuf", struct: str = "fmajor") -> bass.Bass:
    mdt = {
        "f32r": mybir.dt.float32r,
        "bf16": mybir.dt.bfloat16,
        "f16": mybir.dt.float16,
    }[mode]

    nc = bacc.Bacc(
        "TRN2",
        target_bir_lowering=False,
        debug=False,
        num_devices=B,
    )

    # DRAM I/O. Matmul inputs are pre-chunked host-side to [C, P, n] so each
    # chunk DMA is a clean 2D copy and compute can start on chunk 0 early.
    xT = nc.dram_tensor("xT", [C, P, LP], mdt, kind="ExternalInput").ap()
    cw = nc.dram_tensor("cw", [W, C, P, D], mdt, kind="ExternalInput").ap()
    pw = nc.dram_tensor("pw", [C, P, D], mdt, kind="ExternalInput").ap()
    if struct == "fmajor":
        pb = nc.dram_tensor("pb", [D, 1], F32, kind="ExternalInput").ap()
        out = nc.dram_tensor("out", [W, D, L], F32, kind="ExternalOutput").ap()
    else:
        pb = nc.dram_tensor("pb", [P, D], F32, kind="ExternalInput").ap()
        out = nc.dram_tensor("out", [L, W, D], F32, kind="ExternalOutput").ap()

    with tile.TileContext(nc) as tc, ExitStack() as ctx:
        const_pool = ctx.enter_context(tc.tile_pool(name="const", bufs=1))
        cw_pool = ctx.enter_context(tc.tile_pool(name="cw", bufs=2))
        h_pool = ctx.enter_context(tc.tile_pool(name="h", bufs=2))
        out_pool = ctx.enter_context(tc.tile_pool(name="out", bufs=4))
        if cumsum == "psum":
            psc_pool = ctx.enter_context(tc.tile_pool(name="psc", bufs=1, space="PSUM"))
            psp_pool = ctx.enter_context(tc.tile_pool(name="psp", bufs=2, space="PSUM"))
        else:
            psc_pool = ctx.enter_context(tc.tile_pool(name="psc", bufs=4, space="PSUM"))
            psp_pool = ctx.enter_context(tc.tile_pool(name="psp", bufs=4, space="PSUM"))

        if WARMUP:
            # Dummy matmuls with no data dependencies: they run during the
            # initial DMA wait and hold the PE busy >3.4us so the HAM clock
            # gate opens (1.2 -> 2.4 GHz) before the first real matmul. Same
            # dtype/shape class as the real matmuls (fp32 dummies hang the HW).
            wa = const_pool.tile([P, P], mdt, name="warm_a")
            wb = const_pool.tile([P, 512], mdt, name="warm_b")
            nc.gpsimd.memset(wa[:], 0.0)
            nc.gpsimd.memset(wb[:], 0.0)
            for wi in range(WARMUP):
                wp = psc_pool.tile([P, L], F32, tag="psc", name=f"warm_ps{wi}")
                nc.tensor.matmul(
                    wp[:], lhsT=wa[:], rhs=wb[:], start=True, stop=True
                )

        def load_cw(k):
            ts = []
            for c in range(C):
                t = cw_pool.tile([P, D], mdt, tag=f"cw{c}", name=f"cw{c}_{k}")
                nc.sync.dma_start(t[:], cw[k, c, :, :])
                ts.append(t)
            return ts

        # Interleave the startup loads so the first conv matmuls (which need
        # cw[0] chunk c + xT chunk c) can begin as soon as chunk 0 lands.
        cw_cur = []
        xT_t = []
        for c in range(C):
            t = cw_pool.tile([P, D], mdt, tag=f"cw{c}", name=f"cw{c}_0")
            nc.sync.dma_start(t[:], cw[0, c, :, :])
            cw_cur.append(t)
            xt = const_pool.tile([P, LP], mdt, tag=f"xt{c}", name=f"xt{c}")
            nc.sync.dma_start(xt[:], xT[c, :, :])
            xT_t.append(xt)

        pw_t = []
        for c in range(C):
            t = const_pool.tile([P, D], mdt, tag=f"pw{c}", name=f"pw{c}")
            nc.sync.dma_start(t[:], pw[c, :, :])
            pw_t.append(t)
        if struct == "fmajor":
            pb_t = []
            for c in range(C):
                t = const_pool.tile([P, 1], F32, tag=f"pb{c}", name=f"pb{c}")
                nc.sync.dma_start(t[:], pb[c * P : (c + 1) * P, :])
                pb_t.append(t)
        else:
            pb_t = const_pool.tile([P, D], F32)
            nc.sync.dma_start(pb_t[:], pb[:])

        if cumsum == "psum":
            # 6 persistent PSUM banks accumulate the conv cumsum across taps.
            sp_acc = [
                psc_pool.tile([P, L], F32, tag=f"sp{ob}", name=f"sp{ob}")
                for ob in range(C)
            ]
            spans = None
        else:
            spans = const_pool.tile([P, C * L], F32)      # running conv cumsum
            nc.gpsimd.memset(spans[:], 0.0)

        for k in range(W):
            cw_next = load_cw(k + 1) if k + 1 < W else None

            # --- conv tap k: psum[o_blk, l] = sum_d cw^T[d, o] * x^T[d, l+k]
            h_t = [h_pool.tile([P, L], mdt, tag=f"h{c}", name=f"h{c}_{k}") for c in range(C)]
            for ob in range(C):
                if cumsum == "psum":
                    ps = sp_acc[ob]
                    for c in range(C):
                        nc.tensor.matmul(
                            ps[:],
                            lhsT=cw_cur[c][:, ob * P : (ob + 1) * P],
                            rhs=xT_t[c][:, k : k + L],
                            start=(k == 0 and c == 0),
                            stop=(k == W - 1 and c == C - 1),
                            skip_group_check=True,
                        )
                    nc.scalar.activation(h_t[ob][:], ps[:], RELU)
                else:
                    ps = psc_pool.tile([P, L], F32, tag="psc")
                    for c in range(C):
                        nc.tensor.matmul(
                            ps[:],
                            lhsT=cw_cur[c][:, ob * P : (ob + 1) * P],
                            rhs=xT_t[c][:, k : k + L],
                            start=(c == 0),
                            stop=(c == C - 1),
                        )
                    sp = spans[:, ob * L : (ob + 1) * L]
                    nc.vector.tensor_add(sp, sp, ps[:])                    # cumsum
                    nc.scalar.activation(h_t[ob][:], sp, RELU)

            if struct == "fmajor":
                # --- proj tap k (feature-major): out^T[o2_blk, l] =
                #     sum_d pw^T[d, o2] * h^T[d, l]; 36 N=512 matmuls.
                for o2b in range(C):
                    o_t = out_pool.tile([P, L], F32, tag="out", name=f"o_{k}_{o2b}")
                    pp = psp_pool.tile([P, 512], F32, tag="psp", name=f"pp_{k}_{o2b}")
                    for c in range(C):
                        nc.tensor.matmul(
                            pp[:],
                            lhsT=pw_t[c][:, o2b * P : (o2b + 1) * P],
                            rhs=h_t[c][:],
                            start=(c == 0),
                            stop=(c == C - 1),
                        )
                    nc.vector.tensor_scalar_add(o_t[:], pp[:], pb_t[o2b][:])
                    nc.sync.dma_start(out[k, o2b * P : (o2b + 1) * P, :], o_t[:])
            else:
                # --- proj tap k: out[l_blk, o2] = sum_d h^T[d, l]*pw^T[d, o2]+b
                for lb in range(NB):
                    o_t = out_pool.tile([P, D], F32, tag="out")
                    for n0, nn in ((0, 512), (512, 256)):
                        pp = psp_pool.tile([P, 512], F32, tag="psp")
                        for c in range(C):
                            nc.tensor.matmul(
                                pp[:, 0:nn],
                                lhsT=h_t[c][:, lb * P : (lb + 1) * P],
                                rhs=pw_t[c][:, n0 : n0 + nn],
                                start=(c == 0),
                                stop=(c == C - 1),
                            )
                        nc.vector.tensor_add(
                            o_t[:, n0 : n0 + nn], pp[:, 0:nn], pb_t[:, n0 : n0 + nn]
                        )
                    nc.sync.dma_start(out[lb * P : (lb + 1) * P, k, :], o_t[:])

            cw_cur = cw_next

    nc.compile()
    return nc


_program_cache: dict = {}


def _get_program(mode: str, cumsum: str = None, struct: str = None) -> bass.Bass:
    if cumsum is None:
        cumsum = CUMSUM
    if struct is None:
        struct = STRUCT
    key = (mode, cumsum, struct, WARMUP)
    if key not in _program_cache:
        _program_cache[key] = _build_program(mode, cumsum, struct)
    return _program_cache[key]


def _np_dt(mode: str):
    return {"f32r": np.float32, "bf16": ml_dtypes.bfloat16, "f16": np.float16}[mode]


def _prep_inputs(x, conv_w, proj_w, proj_b, mode: str):
    x = np.asarray(x, dtype=np.float32)
    conv_w = np.asarray(conv_w, dtype=np.float32)
    proj_w = np.asarray(proj_w, dtype=np.float32)
    proj_b = np.asarray(proj_b, dtype=np.float32)
    ndt = _np_dt(mode)

    xT_all = np.zeros((B, D, LP), dtype=np.float32)              # [B, D, L+W-1]
    xT_all[:, :, :L] = x.transpose(0, 2, 1)
    xT_all = np.ascontiguousarray(xT_all.reshape(B, C, P, LP).astype(ndt))
    cwT = np.ascontiguousarray(
        conv_w.transpose(2, 1, 0).reshape(W, C, P, D).astype(ndt)
    )                                                            # [W, C, P, o]
    pwT = np.ascontiguousarray(proj_w.T.reshape(C, P, D).astype(ndt))
    if STRUCT == "fmajor":
        pbb = np.ascontiguousarray(proj_b.reshape(D, 1))
    else:
        pbb = np.ascontiguousarray(np.broadcast_to(proj_b[None, :], (P, D)))
    return xT_all, cwT, pwT, pbb


def kernel(x, conv_w, proj_w, proj_b):
    global LAST_RESULTS
    nc = _get_program(MODE, CUMSUM, STRUCT)
    xT_all, cwT, pwT, pbb = _prep_inputs(x, conv_w, proj_w, proj_b, MODE)
    in_maps = [
        {"xT": xT_all[b], "cw": cwT, "pw": pwT, "pb": pbb} for b in range(B)
    ]
    res = bass_utils.run_bass_kernel_spmd(
        nc, in_maps, core_ids=list(range(B)), trace=TRACE
    )
    LAST_RESULTS = res
    if STRUCT == "fmajor":
        # per-core out is [W, D, L]; final layout is [L, W, D]
        return np.stack(
            [np.ascontiguousarray(r["out"].transpose(2, 0, 1)) for r in res.results],
            axis=0,
        )
    return np.stack([r["out"] for r in res.results], axis=0)

